# revision 4
# baseline (speedup 1.0000x reference)
"""Trainium2 Bass kernel for EnhancedGraphSAGE (embed -> 2x SAGE-mean -> GAT -> MLP).

Self-contained: takes full inputs, shards node-wise across 8 NeuronCores
internally, returns the full [N, C] output.

v2 design (vs v1 baseline):
- No embed-table phase: SAGE1 gathers raw x rows (bf16, 256B) from a
  host-provided table and folds Wemb@Wn1 into the post-aggregation matmul;
  the deg>0 bias term is a rank-1 matmul. Saves ~123us of DMA and a serial
  phase.
- AllGathers are chunked (4 chunks) so the collectives and table repacks
  overlap SAGE compute instead of serializing ~240us each.
- PSUM->SBUF copies ride the Activation engine (otherwise idle) to keep DVE
  for the GAT edge-softmax weighting.
- Table row ids are chunk-major (chunk, core, block, p) so the chunked
  AllGather output concatenation IS the table order; host-computed gather
  indices absorb it.
"""

import numpy as np

import concourse.bacc as bacc
import concourse.bass as bass
import concourse.mybir as mybir
import concourse.tile as tile
from concourse.bass_utils import run_bass_kernel_spmd
from concourse.masks import make_identity

# Problem constants (hardcoded per spec)
N, E, IN, H, HEADS, C = 50000, 800000, 128, 64, 4, 40
SLOPE = 0.2

# Sharding geometry
NCORES = 8
NBLK = 56              # dst blocks per core
PB = 128               # dst slots per block
TPH = 8                # gather tiles per half (1024 idx limit of dma_gather)
TPB = 2 * TPH          # tiles per block
SLOTH = TPH * 128      # slots per half
S16 = SLOTH // 16      # idx columns in packed [128, S16] layout
OWN = NBLK * PB        # own nodes per core (7168)
NID = NCORES * OWN     # internal id space (57344)
HALFR = NID // 2       # table half split (28672 < 32768)
D = 128                # table row width (bf16 -> 256B rows)
CH = 512               # dense chunk (nodes per matmul)
NCH_OWN = OWN // CH    # 14

# AllGather chunking: block ranges per chunk
CHUNKS = [(0, 16), (16, 32), (32, 44), (44, 52), (52, 56)]

F32 = mybir.dt.float32
BF16 = mybir.dt.bfloat16
FP8 = mybir.dt.float8e4
I16 = mybir.dt.int16
NP_BF16 = mybir.dt.np(BF16)
NP_FP8 = mybir.dt.np(FP8)

_cached = {}


def _build_bass(upto=99):
    nc = bacc.Bacc("TRN2", target_bir_lowering=False, debug=False,
                   num_devices=NCORES)

    # ---- I/O ----
    xq = nc.dram_tensor("xq", [NID, D], BF16, kind="ExternalInput")
    xob = nc.dram_tensor("xob", [IN, OWN], BF16, kind="ExternalInput")
    idx_in = nc.dram_tensor("idx_in", [NBLK, 128, 2, S16], I16, kind="ExternalInput")
    mask_in = nc.dram_tensor("mask_in", [NBLK, 128, TPB * 128], FP8, kind="ExternalInput")
    maskT_in = nc.dram_tensor("maskT_in", [NBLK, 128, TPB * 128], FP8, kind="ExternalInput")
    dgib_in = nc.dram_tensor("dgib_in", [1, OWN], BF16, kind="ExternalInput")
    m1_in = nc.dram_tensor("m1_in", [1, OWN], BF16, kind="ExternalInput")

    wemb_in = nc.dram_tensor("wemb_in", [IN, H], BF16, kind="ExternalInput")
    bembc = nc.dram_tensor("bembc", [H, 1], F32, kind="ExternalInput")
    ws1_in = nc.dram_tensor("ws1_in", [H, H], BF16, kind="ExternalInput")
    wq1_in = nc.dram_tensor("wq1_in", [IN, H], BF16, kind="ExternalInput")
    u1_in = nc.dram_tensor("u1_in", [1, H], BF16, kind="ExternalInput")
    bn1_in = nc.dram_tensor("bn1_in", [H, 1], F32, kind="ExternalInput")
    ws2_in = nc.dram_tensor("ws2_in", [H, H], BF16, kind="ExternalInput")
    wn2_in = nc.dram_tensor("wn2_in", [H, H], BF16, kind="ExternalInput")
    bn2_in = nc.dram_tensor("bn2_in", [H, 1], F32, kind="ExternalInput")
    wl_in = nc.dram_tensor("wl_in", [H, HEADS], BF16, kind="ExternalInput")
    wr_in = nc.dram_tensor("wr_in", [H, HEADS], BF16, kind="ExternalInput")
    ulo_in = nc.dram_tensor("ulo_in", [128, H], BF16, kind="ExternalInput")
    uhi_in = nc.dram_tensor("uhi_in", [128, H], BF16, kind="ExternalInput")
    b1p = nc.dram_tensor("b1p", [H, 1], F32, kind="ExternalInput")
    w2_in = nc.dram_tensor("w2_in", [H, C], F32, kind="ExternalInput")
    b2c = nc.dram_tensor("b2c", [C, 1], F32, kind="ExternalInput")

    out = nc.dram_tensor("out", [OWN, C], F32, kind="ExternalOutput")

    with tile.TileContext(nc) as tc:
        with (
            tc.tile_pool(name="wpool", bufs=1) as wp,
            tc.tile_pool(name="sbuf", bufs=3) as sb,
            tc.tile_pool(name="big", bufs=1) as bigp,
            tc.tile_pool(name="psum", bufs=2, space="PSUM") as pp,
            tc.tile_pool(name="dram", bufs=1, space="DRAM") as dram,
        ):
            # ---- weights resident in SBUF ----
            w_emb = wp.tile([IN, H], BF16); nc.scalar.dma_start(w_emb[:], wemb_in[:])
            b_embc = wp.tile([H, 1], F32); nc.scalar.dma_start(b_embc[:], bembc[:])
            w_s1 = wp.tile([H, H], BF16); nc.scalar.dma_start(w_s1[:], ws1_in[:])
            w_q1 = wp.tile([IN, H], BF16); nc.scalar.dma_start(w_q1[:], wq1_in[:])
            u_1 = wp.tile([1, H], BF16); nc.scalar.dma_start(u_1[:], u1_in[:])
            b_n1 = wp.tile([H, 1], F32); nc.scalar.dma_start(b_n1[:], bn1_in[:])
            w_s2 = wp.tile([H, H], BF16); nc.scalar.dma_start(w_s2[:], ws2_in[:])
            w_n2 = wp.tile([H, H], BF16); nc.scalar.dma_start(w_n2[:], wn2_in[:])
            b_n2 = wp.tile([H, 1], F32); nc.scalar.dma_start(b_n2[:], bn2_in[:])
            w_lr = wp.tile([H, 2 * HEADS], BF16)
            nc.scalar.dma_start(w_lr[:, 0:HEADS], wl_in[:])
            nc.scalar.dma_start(w_lr[:, HEADS:], wr_in[:])
            u_lo = wp.tile([128, H], BF16); nc.scalar.dma_start(u_lo[:], ulo_in[:])
            u_hi = wp.tile([128, H], BF16); nc.scalar.dma_start(u_hi[:], uhi_in[:])
            b_1p = wp.tile([H, 1], F32); nc.scalar.dma_start(b_1p[:], b1p[:])
            w_2 = wp.tile([H, C], F32); nc.scalar.dma_start(w_2[:], w2_in[:])
            b_2 = wp.tile([C, 1], F32); nc.scalar.dma_start(b_2[:], b2c[:])
            m1_sb = wp.tile([1, OWN], BF16); nc.scalar.dma_start(m1_sb[:], m1_in[:])

            id64b = wp.tile([64, 64], BF16)
            make_identity(nc, id64b[:])
            id128f = wp.tile([128, 128], F32)
            make_identity(nc, id128f[:])
            id128b = wp.tile([128, 128], BF16)
            nc.vector.tensor_copy(id128b[:], id128f[:])
            id40f = wp.tile([40, 40], F32)
            make_identity(nc, id40f[:])

            # deginv replicated across partitions: dgi_rep[p, n] = 1/deg(n)
            # built via PE rank-1 (ones x dgi_row) so the per-dst-column scale
            # of the transposed aggregate is a plain DVE tensor_tensor mul.
            dgi_row = wp.tile([1, OWN], BF16)
            nc.scalar.dma_start(dgi_row[:], dgib_in[:])
            ones1 = wp.tile([1, 128], BF16)
            nc.vector.memset(ones1[:], 1.0)
            dgi_rep = bigp.tile([128, OWN], BF16)
            for ch in range(NCH_OWN):
                pdg = pp.tile([128, CH], F32, space="PSUM", tag="psB", bufs=3)
                nc.tensor.matmul(pdg[:], ones1[:], dgi_row[:, ch * CH:(ch + 1) * CH],
                                 start=True, stop=True)
                nc.scalar.activation(dgi_rep[:, ch * CH:(ch + 1) * CH], pdg[:],
                                     mybir.ActivationFunctionType.Identity)

            # resident masks: first NRES blocks' one-hot tiles stay in SBUF
            NRES = 16
            mask_res = bigp.tile([128, NRES, TPB * 128], FP8)
            for rb in range(NRES):
                nc.scalar.dma_start(mask_res[:, rb, :], mask_in[rb])

            # persistent feature planes (bf16, feat-major)
            h1T = bigp.tile([H, OWN], BF16, tag="hT", bufs=2)
            h2T = bigp.tile([H, OWN], BF16, tag="hT", bufs=2)
            er_all = bigp.tile([128, NBLK, HEADS], BF16)
            og_nm = bigp.tile([128, NBLK, 2 * H * 2], BF16)  # node-major GAT out

            # DRAM: per-layer AG staging (mine -> collective agc -> table)
            # one Shared tensor per AG chunk (tile requires a single writer)
            # fp8 transport+tables for layers 2/3: halves the 40GB/s
            # collective bytes; the 256B gather descriptor then carries 256
            # fp8 columns (first 64/68 valid)
            mine2 = [dram.tile([(b1 - b0) * 128, H], FP8, name=f"mine2_{ci}")
                     for ci, (b0, b1) in enumerate(CHUNKS)]
            agc2 = [dram.tile([(b1 - b0) * 128 * NCORES, H], FP8,
                              addr_space="Shared", name=f"agc2_{ci}")
                    for ci, (b0, b1) in enumerate(CHUNKS)]
            tab2 = dram.tile([NID, 2 * D], FP8)
            mineg = [dram.tile([(b1 - b0) * 128, H + HEADS], FP8,
                               name=f"mineg_{ci}")
                     for ci, (b0, b1) in enumerate(CHUNKS)]
            agcg = [dram.tile([(b1 - b0) * 128 * NCORES, H + HEADS], FP8,
                              addr_space="Shared", name=f"agcg_{ci}")
                    for ci, (b0, b1) in enumerate(CHUNKS)]
            tabg = dram.tile([NID, 2 * D], FP8)

            # ============ own-feature dense embed: h1T = WembT xo + b ========
            for ch in range(NCH_OWN):
                xb = sb.tile([IN, CH], BF16, tag="xb")
                nc.scalar.dma_start(xb[:], xob[:, ch * CH:(ch + 1) * CH])
                ph = pp.tile([H, CH], F32, space="PSUM", tag="psB", bufs=3)
                nc.tensor.matmul(ph[:], w_emb[:], xb[:], start=True, stop=True)
                nc.scalar.activation(h1T[:, ch * CH:(ch + 1) * CH], ph[:],
                                     mybir.ActivationFunctionType.Identity,
                                     bias=b_embc[:], scale=1.0)

            # ============ generic SAGE layer ============
            def sage_layer(layer, hT_in, hT_out, table, w_self, w_neigh_or_q,
                           b_n, mine, agc, tab, width, with_el=False):
                """One SAGE pass: per-block agg + dense + write_rows + chunked AG."""
                it4 = None
                wide = table is xq  # layer 1 aggregates 128-wide raw x
                gdt, gcols = (BF16, D) if wide else (FP8, 2 * D)
                for b in range(NBLK):
                    # --- aggregation ---
                    if b % 4 == 0:
                        it4 = sb.tile([128, 4, 2, S16], I16, tag="it", bufs=2)
                        nc.sync.dma_start(it4[:], idx_in[b:b + 4].rearrange(
                            "q p h s -> p q h s"))
                    if b < NRES:
                        mk = mask_res[:, b, :]
                    else:
                        mkt = sb.tile([128, TPB * 128], FP8, tag="mk", bufs=4)
                        nc.sync.dma_start(mkt[:], mask_in[b])
                        mk = mkt[:]
                    g = sb.tile([128, TPB, gcols], gdt, tag="g", bufs=5)
                    nc.gpsimd.dma_gather(g[:, 0:TPH, :], table[0:HALFR, :],
                                         it4[:, b % 4, 0, :], SLOTH, SLOTH, gcols)
                    nc.gpsimd.dma_gather(g[:, TPH:TPB, :], table[HALFR:NID, :],
                                         it4[:, b % 4, 1, :], SLOTH, SLOTH, gcols)
                    # swapped orientation: pa[f, d] = sum_slots g[slot, f] *
                    # mask[slot, d] -- feat-major aggregate, no transpose stage
                    WA = 128 if wide else H
                    pa = pp.tile([WA, 128], F32, space="PSUM", tag="psAcc", bufs=3)
                    for t in range(TPB):
                        nc.tensor.matmul(pa[:], g[:, t, 0:WA],
                                         mk[:, t * 128:(t + 1) * 128],
                                         start=(t == 0), stop=(t == TPB - 1))
                    bs = slice(b * 128, (b + 1) * 128)
                    nb = sb.tile([WA, 128], BF16, tag="nb")
                    nc.vector.tensor_mul(nb[:], pa[:], dgi_rep[0:WA, bs])
                    # --- dense: h_out = relu(hin@Ws + neigh-term + bias) ---
                    p2 = pp.tile([H, 128], F32, space="PSUM", tag="psB", bufs=3)
                    nc.tensor.matmul(p2[:], w_neigh_or_q[:], nb[:],
                                     start=True, stop=False)
                    last = not (wide and layer == 1)
                    nc.tensor.matmul(p2[:], w_self[:], hT_in[:, bs],
                                     start=False, stop=last)
                    if not last:  # rank-1 deg>0 embed-bias term (layer 1)
                        nc.tensor.matmul(p2[:], u_1[:], m1_sb[0:1, bs],
                                         start=False, stop=True)
                    nc.scalar.activation(hT_out[:, bs], p2[:],
                                         mybir.ActivationFunctionType.Relu,
                                         bias=b_n[:], scale=1.0)
                    # --- write node-major rows for the AllGather ---
                    pw = pp.tile([128, H], BF16, space="PSUM", tag="psT", bufs=2)
                    nc.tensor.transpose(pw[:], hT_out[:, bs], id64b[:])
                    stg = sb.tile([128, width], FP8, tag="stg2")
                    nc.scalar.activation(stg[:, 0:H], pw[:],
                                         mybir.ActivationFunctionType.Identity)
                    if with_el:
                        pelr = pp.tile([128, 2 * HEADS], F32, space="PSUM",
                                       tag="psT", bufs=2)
                        nc.tensor.matmul(pelr[:], hT_out[:, bs], w_lr[:],
                                         start=True, stop=True)
                        nc.scalar.activation(stg[:, H:width], pelr[:, 0:HEADS],
                                             mybir.ActivationFunctionType.Identity)
                        nc.vector.tensor_copy(er_all[:, b, :], pelr[:, HEADS:])
                    ci_b = next(ci for ci, (b0, b1) in enumerate(CHUNKS)
                                if b0 <= b < b1)
                    mb0 = CHUNKS[ci_b][0] * 128
                    nc.sync.dma_start(
                        mine[ci_b][b * 128 - mb0:(b + 1) * 128 - mb0, :], stg[:])
                    # --- chunked AllGather, emitted a few blocks late so the
                    # Pool-queue wait on the chunk's mine writes never
                    # head-of-line-blocks the gather descriptor stream ---
                    for ci, (b0, b1) in enumerate(CHUNKS):
                        if b == min(b1 - 1 + 6, NBLK - 1):
                            nc.gpsimd.collective_compute(
                                "AllGather", mybir.AluOpType.bypass,
                                replica_groups=[list(range(NCORES))],
                                ins=[mine[ci][:]], outs=[agc[ci][:]],
                            )
                # repacks at loop end on the Pool queue: Pool's next work is
                # the next layer's gathers, which need the full table anyway
                # (split in halves: SWDGE DMA caps at 16384 descriptors)
                for ci, (b0, b1) in enumerate(CHUNKS):
                    g0, g1 = b0 * 128 * NCORES, b1 * 128 * NCORES
                    gm = (g0 + g1) // 2
                    rows = (g1 - g0) // 2
                    nc.gpsimd.dma_start(tab[g0:gm, 0:width], agc[ci][0:rows, :])
                    nc.gpsimd.dma_start(tab[gm:g1, 0:width],
                                        agc[ci][rows:2 * rows, :])

            if upto >= 1:
                sage_layer(1, h1T, h2T, xq, w_s1, w_q1, b_n1,
                           mine2, agc2, tab2, H, with_el=False)
            h3T = bigp.tile([H, OWN], BF16, tag="hT", bufs=2)
            if upto >= 2:
                sage_layer(2, h2T, h3T, tab2, w_s2, w_n2, b_n2,
                           mineg, agcg, tabg, H + HEADS, with_el=True)

            # ============ GAT dense + classifier (per 4-block chunk) ========
            def og_stage(ch, half):
                stgT = sb.tile([128, CH], BF16, tag=f"ogs{half}", bufs=2)
                for q in range(4):
                    bq = ch * 4 + q
                    ptg = pp.tile([128, 128], BF16, space="PSUM", tag="psAcc", bufs=3)
                    nc.tensor.transpose(
                        ptg[:], og_nm[:, bq, half * 128:(half + 1) * 128], id128b[:])
                    nc.scalar.activation(stgT[:, q * 128:(q + 1) * 128], ptg[:],
                                         mybir.ActivationFunctionType.Identity)
                return stgT

            def cls_chunk(ch):
                og_loS = og_stage(ch, 0)
                og_hiS = og_stage(ch, 1)
                p4 = pp.tile([H, CH], F32, space="PSUM", tag="psAcc", bufs=3)
                nc.tensor.matmul(p4[:], u_lo[:], og_loS[:],
                                 start=True, stop=False)
                nc.tensor.matmul(p4[:], u_hi[:], og_hiS[:],
                                 start=False, stop=True)
                h4 = sb.tile([H, CH], F32, tag="h4")
                nc.scalar.activation(h4[:], p4[:],
                                     mybir.ActivationFunctionType.Relu,
                                     bias=b_1p[:], scale=1.0)
                plg = pp.tile([C, CH], F32, space="PSUM", tag="psAcc", bufs=3)
                nc.tensor.matmul(plg[:], w_2[:], h4[:], start=True, stop=True)
                lg = sb.tile([C, CH], F32, tag="lg")
                nc.scalar.activation(lg[:], plg[:],
                                     mybir.ActivationFunctionType.Identity,
                                     bias=b_2[:], scale=1.0)
                ostg = sb.tile([128, 4, C], F32, tag="ostg")
                for q in range(4):
                    plt = pp.tile([128, C], F32, space="PSUM", tag="psAcc", bufs=3)
                    nc.tensor.transpose(plt[:], lg[:, q * 128:(q + 1) * 128], id40f[:])
                    nc.vector.tensor_copy(ostg[:, q, :], plt[:])
                nc.sync.dma_start(
                    out[ch * CH:(ch + 1) * CH, :].rearrange("(q p) c -> p q c", p=128),
                    ostg[:])

            # ================= GAT aggregation =================
            it4g = None
            for b in range(NBLK if upto >= 3 else 0):
                if b % 4 == 0:
                    it4g = sb.tile([128, 4, 2, S16], I16, tag="it", bufs=2)
                    nc.sync.dma_start(it4g[:], idx_in[b:b + 4].rearrange(
                        "q p h s -> p q h s"))
                if b < NRES:
                    mk = mask_res[:, b, :]
                else:
                    mkt = sb.tile([128, TPB * 128], FP8, tag="mk", bufs=4)
                    nc.sync.dma_start(mkt[:], mask_in[b])
                    mk = mkt[:]
                mt = sb.tile([128, TPB * 128], FP8, tag="mt", bufs=3)
                nc.sync.dma_start(mt[:], maskT_in[b])
                g = sb.tile([128, TPB, 2 * D], FP8, tag="g", bufs=5)
                nc.gpsimd.dma_gather(g[:, 0:TPH, :], tabg[0:HALFR, :],
                                     it4g[:, b % 4, 0, :], SLOTH, SLOTH, 2 * D)
                nc.gpsimd.dma_gather(g[:, TPH:TPB, :], tabg[HALFR:NID, :],
                                     it4g[:, b % 4, 1, :], SLOTH, SLOTH, 2 * D)
                # er broadcast to edge slots via maskT matmuls
                perb = pp.tile([128, TPB, HEADS], F32, space="PSUM", tag="psT", bufs=2)
                for t in range(TPB):
                    nc.tensor.matmul(perb[:, t, :], mt[:, t * 128:(t + 1) * 128],
                                     er_all[:, b, :], start=True, stop=True)
                # e = leaky_relu(el + er); ex = exp(e)
                ee = sb.tile([128, TPB, HEADS], BF16, tag="ee")
                nc.vector.tensor_add(ee[:], g[:, :, H:H + HEADS], perb[:])
                # leaky_relu in one DVE op: (ee * slope) max ee
                nc.vector.scalar_tensor_tensor(
                    ee[:], ee[:], SLOPE, ee[:],
                    mybir.AluOpType.mult, mybir.AluOpType.max)
                wst = sb.tile([128, TPB, HEADS * H + HEADS], BF16, tag="wst", bufs=2)
                nc.scalar.activation(wst[:, :, HEADS * H:], ee[:],
                                     mybir.ActivationFunctionType.Exp)
                pg = pp.tile([128, HEADS * H + HEADS], F32, space="PSUM", tag="psB", bufs=3)
                # fused per-head weighting: wst[p,t,h,f] = g[p,t,f]*ex[p,t,h]
                # tiles split 11/5 DVE vs Pool (Pool also carries gather
                # desc-gen; DVE the rest of the softmax) to balance engines
                SPL = 12
                for (weng, tsl, nt) in ((nc.vector, slice(0, SPL), SPL),
                                        (nc.gpsimd, slice(SPL, TPB), TPB - SPL)):
                    weng.tensor_mul(
                        wst[:, tsl, 0:HEADS * H].rearrange(
                            "p t (h f) -> p t h f", h=HEADS),
                        g[:, tsl, 0:H].rearrange(
                            "p t (o f) -> p t o f", o=1).to_broadcast(
                                [128, nt, HEADS, H]),
                        wst[:, tsl, HEADS * H:].rearrange(
                            "p t (h o) -> p t h o", o=1).to_broadcast(
                                [128, nt, HEADS, H]))
                for t in range(TPB):
                    nc.tensor.matmul(pg[:], mk[:, t * 128:(t + 1) * 128],
                                     wst[:, t, :], start=(t == 0),
                                     stop=(t == TPB - 1))
                # normalize by z
                zt = sb.tile([128, HEADS], F32, tag="zt")
                nc.vector.tensor_scalar_max(zt[:], pg[:, HEADS * H:], 1e-20)
                zi = sb.tile([128, HEADS], F32, tag="zi")
                nc.vector.reciprocal(zi[:], zt[:])
                nc.vector.tensor_mul(
                    og_nm[:, b, :].rearrange("p (h f) -> p h f", h=HEADS),
                    pg[:, 0:HEADS * H].rearrange("p (h f) -> p h f", h=HEADS),
                    zi[:].to_broadcast([128, HEADS, H]))
                # classifier chunk interleaved as soon as its 4 blocks exist
                if b % 4 == 3 and upto >= 4:
                    cls_chunk(b // 4)

            if upto < 4:
                zo = sb.tile([128, NBLK, C], F32, tag="zo")
                nc.vector.memset(zo[:], 0.0)
                nc.sync.dma_start(
                    out[:].rearrange("(q p) c -> p q c", p=128), zo[:])

    nc.compile()
    return nc


def _tid_of(gid):
    """Chunk-major table row id for a global-internal node id."""
    core = gid // OWN
    local = gid % OWN
    blk = local // 128
    p = local % 128
    tid = np.zeros_like(gid)
    for (b0, b1) in CHUNKS:
        r0 = b0 * 128
        g0 = r0 * NCORES
        cr = (b1 - b0) * 128
        m = (blk >= b0) & (blk < b1)
        tid[m] = g0 + core[m] * cr + (blk[m] - b0) * 128 + p[m]
    return tid


def _plan(src, dst):
    """Host-side graph partitioning. Returns per-core index/mask arrays."""
    src = np.asarray(src).astype(np.int64)
    dst = np.asarray(dst).astype(np.int64)
    for seed in range(64):
        rng = np.random.default_rng(seed)
        perm = rng.permutation(NID)[:N].astype(np.int64)  # orig -> internal gid
        si = perm[src]
        di = perm[dst]
        ti = _tid_of(si)                    # table row of the src
        gblk = di // PB                      # 0..447
        half = (ti >= HALFR).astype(np.int64)
        grp = gblk * 2 + half
        cnt = np.bincount(grp, minlength=NCORES * NBLK * 2)
        if cnt.max() <= SLOTH:
            break
    else:
        raise RuntimeError("could not pack edges into halves; increase NBLK")

    order = np.lexsort((ti, grp))
    g_sorted = grp[order]
    starts = np.zeros(NCORES * NBLK * 2 + 1, np.int64)
    np.cumsum(cnt, out=starts[1:])
    j_in_grp = np.arange(E, dtype=np.int64) - starts[g_sorted]

    e_ti = ti[order]
    e_di = di[order]
    e_half = half[order]
    e_gblk = gblk[order]
    e_core = e_gblk // NBLK
    e_blk = e_gblk % NBLK

    # idx arrays [NCORES, NBLK, 16, 2, S16] then replicated to 128 partitions
    idx16 = np.zeros((NCORES, NBLK, 16, 2, S16), np.int16)
    val = np.where(e_half == 0, e_ti, e_ti - HALFR).astype(np.int16)
    idx16[e_core, e_blk, j_in_grp % 16, e_half, j_in_grp // 16] = val
    idx16 = np.broadcast_to(idx16[:, :, None, :, :, :],
                            (NCORES, NBLK, 8, 16, 2, S16)).reshape(
                                NCORES, NBLK, 128, 2, S16).copy()

    # masks [NCORES, NBLK, 128, TPB*128] fp8: slot (t, p) -> dst col d
    t_of = (e_half * TPH + j_in_grp // 128).astype(np.int64)
    p_of = (j_in_grp % 128).astype(np.int64)
    d_of = (e_di % PB).astype(np.int64)
    m8 = np.zeros((NCORES, NBLK, 128, TPB * 128), np.uint8)
    one_fp8 = np.array(1.0, NP_FP8).view(np.uint8).item()
    m8[e_core, e_blk, p_of, t_of * 128 + d_of] = one_fp8
    mT8 = m8.reshape(NCORES, NBLK, 128, TPB, 128).transpose(0, 1, 4, 3, 2)
    mT8 = np.ascontiguousarray(mT8).reshape(NCORES, NBLK, 128, TPB * 128)

    # deginv per dst slot
    deg = np.bincount(di, minlength=NID).astype(np.float32)
    dgi = (1.0 / np.maximum(deg, 1.0)).reshape(NCORES, NBLK, PB, 1)
    m1 = (deg > 0).astype(np.float32).reshape(NCORES, 1, OWN)

    return perm, idx16, m8.view(NP_FP8), mT8.view(NP_FP8), dgi, m1


def kernel(x, src, dst, W_embed, b_embed, Ws1, Wn1, bn1, Ws2, Wn2, bn2,
           Wg, al, ar, bg, W1, b1, W2, b2):
    x = np.asarray(x, np.float32)
    perm, idx16, m8, mT8, dgi, m1 = _plan(src, dst)

    if "nc" not in _cached:
        _cached["nc"] = _build_bass()
    nc = _cached["nc"]

    # weight preprocessing
    Wemb = np.asarray(W_embed, np.float32)
    bemb = np.asarray(b_embed, np.float32)
    Wn1 = np.asarray(Wn1, np.float32)
    Wg = np.asarray(Wg, np.float32)
    al = np.asarray(al, np.float32)
    ar = np.asarray(ar, np.float32)
    W1 = np.asarray(W1, np.float32)
    WL = np.stack([Wg[:, h * H:(h + 1) * H] @ al[h] for h in range(HEADS)], 1)
    WR = np.stack([Wg[:, h * H:(h + 1) * H] @ ar[h] for h in range(HEADS)], 1)
    b1p = (np.asarray(b1, np.float32) + np.asarray(bg, np.float32) @ W1)
    U = [Wg[:, h * H:(h + 1) * H] @ W1[h * H:(h + 1) * H] for h in range(HEADS)]
    Ulo = np.vstack([U[0], U[1]]).astype(NP_BF16)
    Uhi = np.vstack([U[2], U[3]]).astype(NP_BF16)
    Wq1 = (Wemb @ Wn1).astype(NP_BF16)           # [IN, H]
    u1row = (bemb @ Wn1).reshape(1, H).astype(NP_BF16)

    # x table in chunk-major tid order + own feat-major planes
    gids = np.arange(NID, dtype=np.int64)
    tids = _tid_of(gids)
    xq = np.zeros((NID, D), NP_BF16)
    xq[tids[perm]] = x.astype(NP_BF16)           # row tid(perm[i]) = x[i]
    xT = np.zeros((IN, NID), np.float32)
    xT[:, perm] = x.T

    common = {
        "xq": xq,
        "wemb_in": Wemb.astype(NP_BF16),
        "bembc": bemb.reshape(H, 1),
        "ws1_in": np.asarray(Ws1, np.float32).astype(NP_BF16),
        "wq1_in": Wq1, "u1_in": u1row,
        "bn1_in": np.asarray(bn1, np.float32).reshape(H, 1),
        "ws2_in": np.asarray(Ws2, np.float32).astype(NP_BF16),
        "wn2_in": np.asarray(Wn2, np.float32).astype(NP_BF16),
        "bn2_in": np.asarray(bn2, np.float32).reshape(H, 1),
        "wl_in": WL.astype(NP_BF16), "wr_in": WR.astype(NP_BF16),
        "ulo_in": Ulo, "uhi_in": Uhi,
        "b1p": b1p.reshape(H, 1),
        "w2_in": np.asarray(W2, np.float32),
        "b2c": np.asarray(b2, np.float32).reshape(C, 1),
    }
    in_maps = []
    for c in range(NCORES):
        m = dict(common)
        m["xob"] = np.ascontiguousarray(
            xT[:, c * OWN:(c + 1) * OWN]).astype(NP_BF16)
        m["idx_in"] = np.ascontiguousarray(idx16[c])
        m["mask_in"] = np.ascontiguousarray(m8[c])
        m["maskT_in"] = np.ascontiguousarray(mT8[c])
        m["dgib_in"] = np.ascontiguousarray(
            dgi[c].reshape(1, OWN)).astype(NP_BF16)
        m["m1_in"] = np.ascontiguousarray(m1[c]).astype(NP_BF16)
        in_maps.append(m)

    res = run_bass_kernel_spmd(nc, in_maps, core_ids=list(range(NCORES)))
    full = np.concatenate([res.results[c]["out"] for c in range(NCORES)], 0)
    return full[perm].astype(np.float32)
